# revision 2
# baseline (speedup 1.0000x reference)
"""Single-head causal attention (B=8, T=2048, C=768, H=64) on 8 TRN2 NeuronCores.

Sharding: data-parallel over the batch dim — one batch element per core.

Per-core algorithm (bf16 matmul operands, fp32 PSUM accumulation):
  - inputs fed transposed + pre-cast to bf16 from the host: xT [C, T].
  - exp table preloaded with a dummy activation at t=0; ~10 warmup matmuls
    keep the PE HAM activity monitor busy so the array reaches 2.4 GHz.
  - xT DMA'd as 5 large column-major pieces on the sync HWDGE ring
    (chunks 0-2 / 3-5 of cols [0:512), then 512-col slabs), so QKV matmuls
    start as early as possible and the ring runs at large-transfer rates.
  - qkT [128, T]: rows 0:64 = q^T, 64:128 = k^T (fused [Wq | Wk] weights);
    k^T half shifted to a base-0 tile via SBUF->SBUF DMA (matmul operands
    must share a base partition).
  - vT80 [80, T]: rows 0:64 = v^T from the Wv matmul, row 64 = ones. One
    DMA-XBAR transpose per 512-col group -> v80 [128, 16, 80] natural-layout
    tiles; [v_j | 1] comes out contiguously for the AV stationary operand.
  - attention in S^T layout (keys j on partitions, queries i on free):
    S^T(j-chunk, i-range) = kT_j.T @ qT, 1024-wide column groups. Scale+exp
    fused on ScalarE (PSUM -> SBUF bf16). Causal: only j <= i blocks are
    computed; leading 128-col diagonal block gets an upper-tri mask multiply.
  - AV: out^T [65, group] += [v_j | 1].T @ expS^T_j; row 64 accumulates the
    softmax denominators for free. Each 512-col half is evacuated
    (PSUM -> SBUF fp32) right after its last j-chunk lands and DMA'd out.
  - output is oT [65, T] fp32 (unnormalized + denominators); the host does
    out = (oT[:64] / oT[64:65]).T — no device-side transposes/reciprocal.

No max-subtraction in softmax: scores * C**-0.5 are bounded (|s| < ~3), exp is
safe in fp32, and the result is mathematically identical to jax.nn.softmax.
"""

import ml_dtypes
import numpy as np

import concourse.bass as bass
import concourse.tile as tile
from concourse import bacc, mybir
from concourse.bass import ds, ts
from concourse.masks import make_upper_triangular

B, T, C, H = 8, 2048, 768, 64
P = 128
NCH = C // P          # 6 contraction chunks for QKV
GW = 1024             # attention output column-group width
NG = T // GW          # 2 groups
NT = T // P           # 16 t-chunks
JPG = GW // P         # 8 j-chunks per group
VP = 80               # vT partition count (64 v rows + ones row + pad to 16x)
SCALE = float(C) ** -0.5
N_WARMUP = 10

F32 = mybir.dt.float32
BF16 = mybir.dt.bfloat16
EXP = mybir.ActivationFunctionType.Exp


def _emit(tc: tile.TileContext, ctx, xT, wqk, wv, oT):
    nc = tc.nc

    consts = ctx.enter_context(tc.tile_pool(name="consts", bufs=1))
    xpool = ctx.enter_context(tc.tile_pool(name="x", bufs=1))
    qpool = ctx.enter_context(tc.tile_pool(name="qkv", bufs=1))

    # ---- t=0: input DMAs, exp-table preload, warmup ----
    # xT pieces, column-major: QKV group g needs all 6 C-chunks of its cols.
    xT_sb = xpool.tile([P, NCH, T], BF16)
    xTr = xT.rearrange("(o p) t -> p o t", p=P)
    nc.sync.dma_start(xT_sb[:, 0:3, 0:512], xTr[:, 0:3, 0:512])
    nc.sync.dma_start(xT_sb[:, 3:6, 0:512], xTr[:, 3:6, 0:512])
    for g in range(1, 4):
        sl = ds(g * 512, 512)
        nc.sync.dma_start(xT_sb[:, :, sl], xTr[:, :, sl])

    w_qk = consts.tile([P, NCH, P], BF16)
    nc.scalar.dma_start(w_qk[:], wqk.rearrange("(o p) m -> p o m", p=P))
    w_v = consts.tile([P, NCH, H], BF16)
    nc.scalar.dma_start(w_v[:], wv.rearrange("(o p) m -> p o m", p=P))

    # preload the exp table set so the ~2.7us load overlaps the input DMA
    dummy = consts.tile([P, 1], F32)
    nc.vector.memset(dummy[:], 0.0)
    nc.scalar.activation(dummy[:], dummy[:], EXP)

    tri = consts.tile([P, P], BF16)
    make_upper_triangular(nc, tri[:], val=1.0, diag=True)

    qkT = qpool.tile([P, T], BF16)
    kT = qpool.tile([H, T], BF16)
    vT80 = qpool.tile([VP, T], BF16)
    v80 = qpool.tile([P, NT, VP], BF16)
    nc.vector.memset(vT80[H:VP, :], 0.0)
    nc.vector.memset(vT80[H : H + 1, :], 1.0)

    # warmup tile for dummy matmuls
    dum = qpool.tile([P, 512], BF16)
    nc.vector.memset(dum[:], 0.0)

    # ---- Phase 1: QKV projections + v DMA-XBAR transposes ----
    with tc.tile_pool(name="p1psum", bufs=2, space="PSUM") as pp1:
        for w in range(N_WARMUP):
            dps = pp1.tile([P, 512], F32, tag="qk", name=f"warm_{w}")
            nc.tensor.matmul(dps[:], dum[:, 0:P], dum[:], start=True, stop=True)

        for g in range(4):
            ps = pp1.tile([P, 512], F32, tag="qk")
            for c in range(NCH):
                nc.tensor.matmul(
                    ps[:],
                    w_qk[:, c, :],
                    xT_sb[:, c, ts(g, 512)],
                    start=(c == 0),
                    stop=(c == NCH - 1),
                )
            nc.vector.tensor_copy(qkT[:, ts(g, 512)], ps[:])
            # k^T lives at partitions 64:128; shift to base partition 0
            nc.scalar.dma_start(kT[:, ts(g, 512)], qkT[H:P, ts(g, 512)])

            ps2 = pp1.tile([H, 512], F32, tag="v")
            for c in range(NCH):
                nc.tensor.matmul(
                    ps2[:],
                    w_v[:, c, :],
                    xT_sb[:, c, ts(g, 512)],
                    start=(c == 0),
                    stop=(c == NCH - 1),
                )
            nc.vector.tensor_copy(vT80[0:H, ts(g, 512)], ps2[:])
            # natural-layout [v | 1] tiles via the DMA transpose XBAR
            nc.scalar.dma_start_transpose(
                v80[:, 4 * g : 4 * g + 4, :], vT80[:, ts(g, 512)]
            )

    # ---- Phase 2/3: attention ----
    sp = ctx.enter_context(tc.tile_pool(name="spsum", bufs=2, space="PSUM"))
    op = ctx.enter_context(tc.tile_pool(name="opsum", bufs=2, space="PSUM"))
    pb = ctx.enter_context(tc.tile_pool(name="probs", bufs=6))
    fin = ctx.enter_context(tc.tile_pool(name="fin", bufs=3))

    def emit_probs(g, jj):
        istart = max(g * GW, jj * P)
        n = (g + 1) * GW - istart
        sps = sp.tile([P, GW], F32, tag="s")
        for h in range(0, n, 512):
            nh = min(512, n - h)
            nc.tensor.matmul(
                sps[:, h : h + nh],
                kT[:, ts(jj, P)],
                qkT[0:H, ds(istart + h, nh)],
                start=True,
                stop=True,
            )
        prb = pb.tile([P, GW], BF16, tag="p")
        nc.scalar.activation(prb[:, :n], sps[:, :n], EXP, scale=SCALE)
        if jj >= JPG * g:
            # leading 128 cols are the diagonal block: upper-tri (j<=i) mask
            nc.vector.tensor_mul(out=prb[:, :P], in0=prb[:, :P], in1=tri[:])
        return prb

    def emit_evac(g, ops, hh):
        osb = fin.tile([H + 1, 512], F32, tag="osb", name=f"osb_{g}_{hh}")
        nc.vector.tensor_copy(osb[:], ops[:, ts(hh, 512)])
        nc.scalar.dma_start(oT[:, ds(g * GW + hh * 512, 512)], osb[:])

    pairs = [(g, jj) for g in range(NG) for jj in range(JPG * g + JPG)]
    ops_by_g = {}
    LOOKAHEAD = 2
    prb_queue = [emit_probs(*pairs[i]) for i in range(LOOKAHEAD)]
    for idx, (g, jj) in enumerate(pairs):
        prb = prb_queue.pop(0)
        if idx + LOOKAHEAD < len(pairs):
            prb_queue.append(emit_probs(*pairs[idx + LOOKAHEAD]))

        if jj == 0:
            ops_by_g[g] = op.tile([H + 1, GW], F32, tag="o", name=f"ops_{g}")
        ops = ops_by_g[g]
        istart = max(g * GW, jj * P)
        n = (g + 1) * GW - istart
        ioff = istart - g * GW
        # split at the ops tile's absolute 512-col PSUM bank boundaries
        seg = ioff
        while seg < ioff + n:
            seg_end = min(ioff + n, (seg // 512 + 1) * 512)
            half = seg // 512
            # last j-chunk writing this 512-wide half of the group
            jj_last = min(JPG * g + JPG - 1, JPG * g + 4 * (half + 1) - 1)
            nc.tensor.matmul(
                ops[:, seg:seg_end],
                v80[:, jj, 0 : H + 1],
                prb[:, seg - ioff : seg_end - ioff],
                start=(jj == 0),
                stop=(jj == jj_last),
            )
            if jj == jj_last:
                # this half is complete: evacuate + stream out immediately
                emit_evac(g, ops, half)
            seg = seg_end


def build():
    from contextlib import ExitStack

    nc = bacc.Bacc("TRN2", target_bir_lowering=False, debug=False, num_devices=B)
    xT = nc.dram_tensor("xT", [C, T], BF16, kind="ExternalInput").ap()
    wqk = nc.dram_tensor("wqk", [C, P], BF16, kind="ExternalInput").ap()
    wv = nc.dram_tensor("wv", [C, H], BF16, kind="ExternalInput").ap()
    oT = nc.dram_tensor("oT", [H + 1, T], F32, kind="ExternalOutput").ap()
    with tile.TileContext(nc) as tc, ExitStack() as ctx:
        _emit(tc, ctx, xT, wqk, wv, oT)
    nc.compile()
    return nc


_NC = None


def _get_nc():
    global _NC
    if _NC is None:
        _NC = build()
    return _NC


def make_in_maps(x, Wk, Wq, Wv):
    bf = ml_dtypes.bfloat16
    wqk = np.ascontiguousarray(np.concatenate([Wq, Wk], axis=1)).astype(bf)
    wv = np.ascontiguousarray(np.asarray(Wv)).astype(bf)
    return [
        {
            "xT": np.ascontiguousarray(np.asarray(x[b]).T).astype(bf),
            "wqk": wqk,
            "wv": wv,
        }
        for b in range(B)
    ]


def finalize_host(oT):
    """oT [65, T] fp32 -> normalized [T, H] output."""
    return np.ascontiguousarray((oT[:H] / oT[H : H + 1]).T, dtype=np.float32)


def kernel(x, Wk, Wq, Wv):
    from concourse.bass_utils import run_bass_kernel_spmd

    nc = _get_nc()
    in_maps = make_in_maps(x, Wk, Wq, Wv)
    r = run_bass_kernel_spmd(nc, in_maps, core_ids=list(range(B)))
    out = np.stack([finalize_host(r.results[b]["oT"]) for b in range(B)])
    return np.ascontiguousarray(out, dtype=np.float32)


# revision 5
# speedup vs baseline: 1.1692x; 1.1692x over previous
"""Single-head causal attention (B=8, T=2048, C=768, H=64) on 8 TRN2 NeuronCores.

Sharding: data-parallel over the batch dim — one batch element per core.

Per-core algorithm (bf16 matmul operands, fp32 PSUM accumulation):
  - inputs fed transposed + pre-cast to bf16 from the host: xT [C, T].
  - engine discipline: ScalarE runs ONLY the exp activations (the softmax is
    ScalarE-bound); sync-HWDGE carries the bulk x input (4 col-major 768KB
    pieces at large-transfer rates); gpsimd-SWDGE carries weights, k^T
    partition shifts and output evacuation DMAs (idle engine, latency
    tolerant); VectorE does PSUM casts, masks and evac copies.
  - exp table preloaded with a dummy activation at t=0; warmup matmuls keep
    the PE HAM activity monitor busy so the array reaches 2.4 GHz.
  - qkT [128, T]: rows 0:64 = q^T, 64:128 = k^T (fused [Wq | Wk] weights);
    k^T shifted to a base-0 tile via gpsimd SBUF->SBUF DMA per 512-col group.
  - vT80 [80, T]: rows 0:64 = v^T, row 64 = ones, rows 65:80 zero. Natural
    [v_j | 1] tiles v80 [128, 16, 80]: chunks 0..7 via PE transposes (before
    attention starts), chunks 8..15 via sync-ring DMA-XBAR transposes (the
    ring is drained by then; those chunks aren't needed until late).
  - QKV for column groups 2,3 is interleaved INTO the attention pair stream
    (single shared PSUM bank) so attention over group 0 starts as soon as
    x cols [0:1024) and their QKV land.
  - attention in S^T layout (keys j on partitions, queries i on free):
    S^T(j-chunk, i-range) = kT_j.T @ qT, 1024-wide column groups. Scale+exp
    fused on ScalarE (PSUM -> SBUF bf16). Causal: only j <= i blocks are
    computed; leading 128-col diagonal block gets an upper-tri mask multiply.
  - AV: out^T [65, half] += [v_j | 1].T @ expS^T_j per 512-col half (own PSUM
    bank); row 64 accumulates softmax denominators. Each half is evacuated
    (PSUM -> SBUF fp32) right after its last j-chunk and DMA'd out.
  - output is oT [65, T] fp32 (unnormalized + denominators); the host does
    out = (oT[:64] / oT[64:65]).T — no device-side transposes/reciprocal.

No max-subtraction in softmax: scores * C**-0.5 are bounded (|s| < ~3), exp is
safe in fp32, and the result is mathematically identical to jax.nn.softmax.
"""

import ml_dtypes
import numpy as np

import concourse.bass as bass
import concourse.tile as tile
from concourse import bacc, mybir
from concourse.bass import ds, ts
from concourse.masks import make_identity, make_upper_triangular

B, T, C, H = 8, 2048, 768, 64
P = 128
NCH = C // P          # 6 contraction chunks for QKV
GW = 1024             # attention output column-group width
NG = T // GW          # 2 groups
NT = T // P           # 16 t-chunks
JPG = GW // P         # 8 j-chunks per group
VP = 80               # vT partition count (64 v rows + ones row + pad to 16x)
SCALE = float(C) ** -0.5
N_WARMUP = 6

F32 = mybir.dt.float32
BF16 = mybir.dt.bfloat16
EXP = mybir.ActivationFunctionType.Exp


def _emit(tc: tile.TileContext, ctx, xT, wqk, wv, oT):
    nc = tc.nc

    consts = ctx.enter_context(tc.tile_pool(name="consts", bufs=1))
    xpool = ctx.enter_context(tc.tile_pool(name="x", bufs=1))
    qpool = ctx.enter_context(tc.tile_pool(name="qkv", bufs=1))

    # ---- t=0: input DMAs, exp-table preload, warmup ----
    xT_sb = xpool.tile([P, NCH, T], BF16)
    xTr = xT.rearrange("(o p) t -> p o t", p=P)
    for g in range(4):
        sl = ds(g * 512, 512)
        nc.sync.dma_start(xT_sb[:, :, sl], xTr[:, :, sl])

    w_qk = consts.tile([P, NCH, P], BF16)
    nc.gpsimd.dma_start(w_qk[:], wqk.rearrange("(o p) m -> p o m", p=P))
    w_v = consts.tile([P, NCH, H], BF16)
    nc.gpsimd.dma_start(w_v[:], wv.rearrange("(o p) m -> p o m", p=P))

    # preload the exp table set so the ~2.7us load overlaps the input DMA
    dummy = consts.tile([P, 1], F32)
    nc.vector.memset(dummy[:], 0.0)
    nc.scalar.activation(dummy[:], dummy[:], EXP)

    # warmup tile for dummy matmuls (memset FIRST on vector: warmup gates PE)
    dum = qpool.tile([P, 512], BF16)
    nc.vector.memset(dum[:], 0.0)

    ident = consts.tile([H, H], BF16)
    make_identity(nc, ident[:])
    tri = consts.tile([P, P], BF16)
    make_upper_triangular(nc, tri[:], val=1.0, diag=True)

    qkT = qpool.tile([P, T], BF16)
    kT = qpool.tile([H, T], BF16)
    vT80 = qpool.tile([VP, T], BF16)
    v80 = qpool.tile([P, NT, VP], BF16)
    # ones column for the PE-transposed chunks 0..7
    nc.vector.memset(v80[:, 0:8, H : H + 1], 1.0)
    # ones row + zero pad rows feeding the XBAR transposes (chunks 8..15)
    nc.vector.memset(vT80[H:VP, GW:T], 0.0)
    nc.vector.memset(vT80[H : H + 1, GW:T], 1.0)

    pp1 = tc.tile_pool(name="p1psum", bufs=2, space="PSUM")
    pq = pp1.__enter__()

    for w in range(N_WARMUP):
        dps = pq.tile([P, 512], F32, tag="qk", name=f"warm_{w}")
        nc.tensor.matmul(dps[:], dum[:, 0:P], dum[:], start=True, stop=True)

    def emit_qk(g, pool, tag="qk"):
        ps = pool.tile([P, 512], F32, tag=tag, name=f"qk_{g}")
        for c in range(NCH):
            nc.tensor.matmul(
                ps[:],
                w_qk[:, c, :],
                xT_sb[:, c, ts(g, 512)],
                start=(c == 0),
                stop=(c == NCH - 1),
            )
        nc.vector.tensor_copy(qkT[:, ts(g, 512)], ps[:])
        # k^T lives at partitions 64:128; shift to base partition 0
        nc.gpsimd.dma_start(kT[:, ts(g, 512)], qkT[H:P, ts(g, 512)])

    def emit_v(g, pool, tag="v"):
        pst = pool.tile([P, 512], F32, tag=tag, name=f"v_{g}")
        ps2 = pst[0:H, :]
        for c in range(NCH):
            nc.tensor.matmul(
                ps2,
                w_v[:, c, :],
                xT_sb[:, c, ts(g, 512)],
                start=(c == 0),
                stop=(c == NCH - 1),
            )
        nc.vector.tensor_copy(vT80[0:H, ts(g, 512)], ps2)

    def emit_pe_transposes(g):
        for t in range(4 * g, 4 * g + 4):
            pt = pq.tile([P, H], BF16, tag="vt", name=f"vt_{t}")
            nc.tensor.transpose(pt[:], vT80[0:H, ts(t, P)], ident[:])
            nc.vector.tensor_copy(v80[:, t, 0:H], pt[:])

    # groups 0,1: qk first (gates attention), then v + PE transposes
    emit_qk(0, pq)
    emit_qk(1, pq)
    emit_v(0, pq)
    emit_pe_transposes(0)
    emit_v(1, pq)
    emit_pe_transposes(1)

    pp1.__exit__(None, None, None)

    # ---- attention (with QKV groups 2,3 interleaved into the pair stream) ----
    sp = ctx.enter_context(tc.tile_pool(name="spsum", bufs=2, space="PSUM"))
    op = ctx.enter_context(tc.tile_pool(name="opsum", bufs=3, space="PSUM"))
    qp2 = ctx.enter_context(tc.tile_pool(name="q2psum", bufs=1, space="PSUM"))
    pb = ctx.enter_context(tc.tile_pool(name="probs", bufs=6))
    fin = ctx.enter_context(tc.tile_pool(name="fin", bufs=3))

    def emit_probs(g, jj):
        istart = max(g * GW, jj * P)
        n = (g + 1) * GW - istart
        sps = sp.tile([P, GW], F32, tag="s")
        for h in range(0, n, 512):
            nh = min(512, n - h)
            nc.tensor.matmul(
                sps[:, h : h + nh],
                kT[:, ts(jj, P)],
                qkT[0:H, ds(istart + h, nh)],
                start=True,
                stop=True,
            )
        prb = pb.tile([P, GW], BF16, tag="p")
        nc.scalar.activation(prb[:, :n], sps[:, :n], EXP, scale=SCALE)
        if jj >= JPG * g:
            # leading 128 cols are the diagonal block: upper-tri (j<=i) mask
            nc.vector.tensor_mul(out=prb[:, :P], in0=prb[:, :P], in1=tri[:])
        return prb

    def emit_evac(g, hh, oph, last):
        osb = fin.tile([H + 1, 512], F32, tag="osb", name=f"osb_{g}_{hh}")
        nc.vector.tensor_copy(osb[:], oph[:])
        eng = nc.sync if last else nc.gpsimd
        eng.dma_start(oT[:, ds(g * GW + hh * 512, 512)], osb[:])

    # deferred QKV/transpose work, interleaved after early attention pairs
    def emit_qkv_late(g):
        emit_qk(g, qp2, tag="qk2")
        emit_v(g, qp2, tag="qk2")
        # natural [v|1] tiles for chunks 4g..4g+4 via the sync-ring DMA XBAR
        nc.sync.dma_start_transpose(
            v80[:, 4 * g : 4 * g + 4, :], vT80[:, ts(g, 512)]
        )

    deferred = {2: lambda: emit_qkv_late(2), 4: lambda: emit_qkv_late(3)}

    pairs = [(g, jj) for g in range(NG) for jj in range(JPG * g + JPG)]
    ops_by_gh = {}
    LOOKAHEAD = 3
    prb_queue = [emit_probs(*pairs[i]) for i in range(LOOKAHEAD)]
    for idx, (g, jj) in enumerate(pairs):
        prb = prb_queue.pop(0)
        if idx + LOOKAHEAD < len(pairs):
            prb_queue.append(emit_probs(*pairs[idx + LOOKAHEAD]))

        if jj == 0:
            for hh in range(2):
                ops_by_gh[(g, hh)] = op.tile(
                    [H + 1, 512], F32, tag="o", name=f"ops_{g}_{hh}"
                )
        istart = max(g * GW, jj * P)
        n = (g + 1) * GW - istart
        ioff = istart - g * GW
        seg = ioff
        while seg < ioff + n:
            seg_end = min(ioff + n, (seg // 512 + 1) * 512)
            half = seg // 512
            # last j-chunk writing this 512-wide half of the group
            jj_last = min(JPG * g + JPG - 1, JPG * g + 4 * (half + 1) - 1)
            oph = ops_by_gh[(g, half)]
            nc.tensor.matmul(
                oph[:, seg - half * 512 : seg_end - half * 512],
                v80[:, jj, 0 : H + 1],
                prb[:, seg - ioff : seg_end - ioff],
                start=(jj == 0),
                stop=(jj == jj_last),
            )
            if jj == jj_last:
                emit_evac(g, half, oph, last=(idx == len(pairs) - 1))
            seg = seg_end

        if idx in deferred:
            deferred[idx]()


def build():
    from contextlib import ExitStack

    nc = bacc.Bacc("TRN2", target_bir_lowering=False, debug=False, num_devices=B)
    xT = nc.dram_tensor("xT", [C, T], BF16, kind="ExternalInput").ap()
    wqk = nc.dram_tensor("wqk", [C, P], BF16, kind="ExternalInput").ap()
    wv = nc.dram_tensor("wv", [C, H], BF16, kind="ExternalInput").ap()
    oT = nc.dram_tensor("oT", [H + 1, T], F32, kind="ExternalOutput").ap()
    with tile.TileContext(nc) as tc, ExitStack() as ctx:
        _emit(tc, ctx, xT, wqk, wv, oT)
    nc.compile()
    return nc


_NC = None


def _get_nc():
    global _NC
    if _NC is None:
        _NC = build()
    return _NC


def make_in_maps(x, Wk, Wq, Wv):
    bf = ml_dtypes.bfloat16
    wqk = np.ascontiguousarray(np.concatenate([Wq, Wk], axis=1)).astype(bf)
    wv = np.ascontiguousarray(np.asarray(Wv)).astype(bf)
    return [
        {
            "xT": np.ascontiguousarray(np.asarray(x[b]).T).astype(bf),
            "wqk": wqk,
            "wv": wv,
        }
        for b in range(B)
    ]


def finalize_host(oT):
    """oT [65, T] fp32 -> normalized [T, H] output."""
    return np.ascontiguousarray((oT[:H] / oT[H : H + 1]).T, dtype=np.float32)


def kernel(x, Wk, Wq, Wv):
    from concourse.bass_utils import run_bass_kernel_spmd

    nc = _get_nc()
    in_maps = make_in_maps(x, Wk, Wq, Wv)
    r = run_bass_kernel_spmd(nc, in_maps, core_ids=list(range(B)))
    out = np.stack([finalize_host(r.results[b]["oT"]) for b in range(B)])
    return np.ascontiguousarray(out, dtype=np.float32)
